# revision 3
# baseline (speedup 1.0000x reference)
"""Multi-head attention kernel for Trainium2 (Bass/Tile), 8-core SPMD — v3.

Problem: B=4, Q=K=2048, C=128, H=8, D=16 attention.
Sharding: core = (batch b, head-group hg): 4 batches x 2 groups of 4 heads.
Core output: out[b, :, 4*hg:4*hg+4, :] as [2048, 64].

v3 design (~205us vs v2 at ~374us):
  - Main loop batched per kt: 16 score MMs (2 per chunk: [2 heads, 256 q] =
    512 moving cols against the strip-packed kT tile; per-head separation
    via zero-strip masking), then 32 col-tiled AV MMs per kt.
  - Per-chunk exp with a 2-slot PSUM score ring: ACT (native exp) and DVE
    (Schraudolph: uint16(1024*x + bias) bitcast fp16; the saturating HW
    cast gives clean underflow->0) run concurrently on alternating chunks.
  - One PSUM tile PER ring slot and per (engine, kt-buffer) e tile: the
    tile framework tracks hazards at tile granularity, so tile-splitting
    is what lets S MMs / ACT exp / DVE exp actually pipeline.
  - AV PSUM protocol: pre-zero avbank with full-bank matmuls (start=True
    lazily zeroes the WHOLE 2KB bank), then all AV MMs accumulate with
    start=False. Epilogue drains wait the bank-pair's last AV (PSUM
    PE-write/engine-read same-bank collision is fatal).
  - Prologue: batched DMA (4 tiles per descriptor, sync+scalar queues),
    f32 PE transposes + single-engine fp16 copies through a bank-granular
    sbank staging ring whose final users fully rewrite each bank (no
    dangling PSUM pending-zero), projections after.
  - Epilogue: kt-outer tail, pipelined PE transposes + DVE recip/mult.

Known-bad on HW (do not re-enable blindly): post-compile LDWEIGHTS dedup
(V3_DEDUP=1) relies on PE weight residency that breaks under this
overlapped schedule despite passing CoreSim.
"""

import math
import os
import sys
from contextlib import ExitStack

import numpy as np

try:
    import concourse.bass as bass
except ImportError:  # container staging path
    sys.path.insert(0, "/opt/trn_rl_repo")
    import concourse.bass as bass

import concourse.bacc as bacc
import concourse.tile as tile
from concourse import mybir
from concourse.bass import _add_dep_helper
from concourse.bass_utils import run_bass_kernel_spmd

B, Q, KS, C, H, D = 4, 2048, 2048, 128, 8, 16
HPC = 4
N_CORES = 8
P = 128
QB = 256
NQB = Q // QB  # 8 chunks per kt
NKT_FULL = KS // P  # 16
NKT = int(os.environ.get("V3_NKT", NKT_FULL))  # truncated debug builds
NCH = NKT * NQB
F32 = mybir.dt.float32
F16 = mybir.dt.float16
U16 = mybir.dt.uint16
LN2 = math.log(2.0)
SCALE_Q = (1.0 / LN2) / math.sqrt(D)  # fold log2e into wq: scores in 2^x domain
ESHIFT = 2.0  # common 2^-ESHIFT factor: fp16/uint16 headroom (cancels in softmax)
SCH_BIAS = 15360.0 - 1024.0 * ESHIFT - 46.0  # fp16 exp bias + mid-rounding corr

# per-kt exp engine split: ACT ~997ns/chunk vs DVE ~1192ns/chunk
ACT_ONLY = os.environ.get("ACT_ONLY", "0") == "1"
_PROBE = []


def exp_assignment(kt):
    if ACT_ONLY:
        return ["act"] * NQB
    # 4.5 / 3.5 average split, alternating by kt parity
    if kt % 2 == 0:
        return ["act", "dve", "act", "dve", "act", "dve", "act", "act"]
    return ["act", "dve", "act", "dve", "act", "dve", "act", "dve"]


def _dep(inst, on, reason="dep"):
    _add_dep_helper(inst.ins, on.ins, sync=True, reason=reason)


def _after(insts, anchor, reason="order"):
    for i in insts:
        _add_dep_helper(i.ins, anchor.ins, sync=False, reason=reason)


def _legalize_waits(nc: bass.Bass) -> None:
    """Move excess embedded semaphore waits onto same-engine sequencer NOPs
    (TRN2 encodings carry at most one wait)."""
    nid = [0]
    for fn in nc.m.functions:
        for blk in fn.blocks:
            out = []
            changed = False
            for inst in blk.instructions:
                si = inst.sync_info
                if (
                    si is not None
                    and si.on_wait
                    and len(si.on_wait) > 1
                    and not (
                        inst.is_sequencer_only()
                        if callable(inst.is_sequencer_only)
                        else inst.is_sequencer_only
                    )
                ):
                    for w in si.on_wait:
                        nop = mybir.InstNoOp(name=f"W-{nid[0]}", ins=[], outs=[])
                        nid[0] += 1
                        nop.engine = inst.engine
                        nop.sync_info = mybir.SyncInfo(on_wait=[w], on_update=[])
                        nc.register_instruction(nop, overwrite=True)
                        out.append(nop)
                    inst.sync_info = mybir.SyncInfo(
                        on_wait=[], on_update=list(si.on_update)
                    )
                    changed = True
                out.append(inst)
            if changed:
                blk.instructions = out


def _dedup_ldweights(nc: bass.Bass) -> int:
    """Delete PE Ldweights that reload the exact weights already resident
    (same AP signature + tile position/size). HW keeps weights resident
    between matmuls; the paired Matmult instructions are non-self-loading
    after compile()'s split pass."""
    deleted = 0
    nid = [0]

    def colrange(pos, siz):
        if pos is None or siz is None:
            return (0, 128)
        return (pos[1], pos[1] + siz[1])

    for fn in nc.m.functions:
        for blk in fn.blocks:
            out = []
            resident = {}  # key: (pos, size) strings -> (ap sig, col range)
            for inst in blk.instructions:
                op = inst.concise_opcode()
                if op == "Ldweights":
                    pos = getattr(inst, "tile_position", None)
                    siz = getattr(inst, "tile_size", None)
                    key = (str(pos), str(siz))
                    cr = colrange(pos, siz)
                    sig = (
                        str(inst.ins[0]),
                        str(getattr(inst, "perf_mode", None)),
                        str(getattr(inst, "is_transpose", None)),
                    )
                    cur = resident.get(key)
                    if cur is not None and cur[0] == sig:
                        # redundant reload: drop, keep its sync on a NOP
                        si = inst.sync_info
                        if si is not None and (si.on_wait or si.on_update):
                            nop = mybir.InstNoOp(
                                name=f"LWD-{nid[0]}", ins=[], outs=[]
                            )
                            nid[0] += 1
                            nop.engine = inst.engine
                            nop.sync_info = si
                            nc.register_instruction(nop, overwrite=True)
                            out.append(nop)
                        deleted += 1
                        continue
                    # new load: invalidate entries whose col range overlaps
                    resident = {
                        k: v
                        for k, v in resident.items()
                        if v[1][1] <= cr[0] or v[1][0] >= cr[1]
                    }
                    resident[key] = (sig, cr)
                    out.append(inst)
                else:
                    out.append(inst)
            blk.instructions = out
    return deleted


def build_attention_nc() -> bass.Bass:
    nc = bacc.Bacc()
    qx_d = nc.dram_tensor("qx", [Q, C], F32, kind="ExternalInput")
    kvx_d = nc.dram_tensor("kvx", [KS, C], F32, kind="ExternalInput")
    wq_d = nc.dram_tensor("wq", [HPC * D, C], F32, kind="ExternalInput")
    wk_d = nc.dram_tensor("wk", [HPC * D, C], F32, kind="ExternalInput")
    wv_d = nc.dram_tensor("wv", [HPC * D, C], F32, kind="ExternalInput")
    out_d = nc.dram_tensor("out", [Q, HPC * D], F32, kind="ExternalOutput")
    DUMP = os.environ.get("V3_DUMP", "0") == "1"
    if DUMP:
        dkT = nc.dram_tensor("dkT", [P, KS], F32, kind="ExternalOutput")
        dqT = nc.dram_tensor("dqT", [P, HPC * Q], F32, kind="ExternalOutput")
        dv = nc.dram_tensor("dv", [P, NKT_FULL * HPC * 32], F32, kind="ExternalOutput")
        de = nc.dram_tensor("de", [P, 5 * HPC * QB], F32, kind="ExternalOutput")
        do = nc.dram_tensor("do", [P, 4 * QB], F32, kind="ExternalOutput")
        dr = nc.dram_tensor("dr", [P, NQB * 2 * HPC], F32, kind="ExternalOutput")

    with tile.TileContext(nc) as tc, ExitStack() as ctx:
        const = ctx.enter_context(tc.tile_pool(name="const", bufs=1))
        sbig = ctx.enter_context(tc.tile_pool(name="sbig", bufs=1))
        psum = ctx.enter_context(tc.tile_pool(name="psum", bufs=1, space="PSUM"))

        # ---- persistent PSUM: score ring (2 slots x 2 banks) + AV (4 banks) ----
        # One tile per ring slot: the tile framework tracks hazards at tile
        # granularity, so slot-separate tiles are what make the S/exp ring
        # actually pipeline (a single tile serializes S MMs vs both exps).
        sb = [psum.tile([P, HPC, QB], F32, name=f"sb{i}") for i in range(2)]
        avbank = psum.tile([P, NQB, QB], F32)

        identity = const.tile([P, P], F32)
        id_ms = nc.gpsimd.memset(identity, 0.0)
        id_sel = nc.gpsimd.affine_select(
            out=identity,
            in_=identity,
            compare_op=mybir.AluOpType.not_equal,
            fill=1.0,
            base=0,
            pattern=[[-1, P]],
            channel_multiplier=1,
        )
        zbias = const.tile([P, 1], F32)
        nc.vector.memset(zbias, -ESHIFT * LN2)
        wabs = const.tile([1, 16], F16)  # PE LDW-absorber source
        nc.vector.memset(wabs, 0.0)
        zmm_w = const.tile([P, P], F16)  # zero stationary for avbank pre-zero
        zw_ms = nc.vector.memset(zmm_w, 0.0)
        zmm_x = const.tile([P, 512], F16)
        zx_ms = nc.vector.memset(zmm_x, 0.0)
        _abs_ctr = [0]

        def pe_abs(*waits, after=None):
            """Real-op absorber: standalone 1x1 fp16 LDWEIGHTS chain, one
            foreign tick per op (scheduler drops bare NOPs; identical APs get
            merged, so each absorber uses a unique source column). Safe only
            where the next matmul reloads its own weights; `after` pins the
            chain in PE program order so it cannot clobber resident weights."""
            ops = []
            for w in waits:
                i = _abs_ctr[0] % 16
                _abs_ctr[0] += 1
                ld = nc.tensor.ldweights(weights=wabs[0:1, i : i + 1])
                _dep(ld, w)
                if ops:
                    _after([ld], ops[-1])
                elif after is not None:
                    _after([ld], after)
                ops.append(ld)
            return ops[-1]

        # ---- SBUF ----
        x_sb = sbig.tile([P, 2, NKT_FULL, P], F32)  # [s, (q|kv), tile, c] staging
        xT_sb = sbig.tile([P, 2, NKT_FULL, P], F16)  # [c, (q|kv), tile, s]
        w_sb = sbig.tile([HPC * D, 3, C], F32)
        wq_pack4 = sbig.tile([P, HPC, HPC, 32], F16)  # [c, variant, strip, 32]
        wk_pack = sbig.tile([P, HPC, 32], F16)
        wv_ext = sbig.tile([P, HPC, 17], F16)
        qTm = sbig.tile([P, HPC, Q], F16)  # per-head masked, scaled log2e/sqrt(D)
        kT = sbig.tile([P, KS], F16)
        v_sb = sbig.tile([P, NKT_FULL, HPC, 32], F16)
        # e tiles: one per (engine, kt-buffer) so every tile has a single
        # writer engine (WAW stays on that engine's FIFO, ACT/DVE exps run
        # concurrently).
        e_act = [sbig.tile([P, 5, HPC, QB], F16, name=f"e_act{i}") for i in range(2)]
        e_dve = [sbig.tile([P, 4, HPC, QB], F16, name=f"e_dve{i}") for i in range(2)]
        o_ev = sbig.tile([P, 4, QB], F32)  # even qb drains (ACT)
        o_od = sbig.tile([P, 4, QB], F32)  # odd qb drains (DVE)
        r_sb = sbig.tile([P, NQB, 2, HPC], F32)
        ofin = sbig.tile([P, NQB, 2, HPC * D], F32)

        # ---------------- stage 0: DMA in (batched, 2 queues) ----------------
        wdmas = [
            nc.sync.dma_start(out=w_sb[:, 0, :], in_=wq_d[:, :]),
            nc.sync.dma_start(out=w_sb[:, 1, :], in_=wk_d[:, :]),
            nc.sync.dma_start(out=w_sb[:, 2, :], in_=wv_d[:, :]),
        ]
        # x DMAs: 4 tiles (512 rows) per descriptor; kv on sync, q on scalar
        kv_dmas = []
        q_dmas = []
        for i in range(4):
            src = kvx_d[512 * i : 512 * (i + 1), :].rearrange(
                "(t p) c -> p t c", p=P
            )
            kv_dmas.append(
                nc.sync.dma_start(out=x_sb[:, 1, 4 * i : 4 * i + 4, :], in_=src)
            )
        for i in range(4):
            src = qx_d[512 * i : 512 * (i + 1), :].rearrange("(t p) c -> p t c", p=P)
            q_dmas.append(
                nc.scalar.dma_start(out=x_sb[:, 0, 4 * i : 4 * i + 4, :], in_=src)
            )

        # zero-fill packed weight regions
        z0 = nc.vector.memset(wq_pack4, 0.0)
        z1 = nc.vector.memset(wk_pack, 0.0)
        z2 = nc.vector.memset(wv_ext, 0.0)
        z3 = nc.vector.memset(v_sb[:, :, :, 17:32], 0.0)

        # ---- weight transposes (avbank even-slot bank ring: banks 0,1,2) ----
        # ---- unified prologue PSUM staging ring over the 4 sbank banks ----
        # PSUM rules honored here:
        #  * PE-write + DVE/ACT-read of the SAME bank is fatal -> every ring
        #    region is one full bank, strictly sequenced producer (PE) ->
        #    consumer (copy) -> next producer (WAR dep 4 jobs back).
        #  * matmul start=True lazily zeroes its WHOLE bank; partial-bank
        #    staging leaves dangling pending-zero -> the ring's last users
        #    (kT/qTm projections) write full banks, and avbank is never
        #    staged on so the AV accumulation sees clean state.
        a0 = pe_abs(id_sel, wdmas[-1])
        id64 = identity[0 : HPC * D, 0 : HPC * D]
        ring_last = []  # per job: last consumer of that bank

        def ring_reg(j):
            sl, hb = divmod(j % 4, 2)
            return sb[sl][:, 2 * hb : 2 * hb + 2, :].rearrange("p a b -> p (a b)")

        # jobs 0..2: w transposes + packing moves (ACT)
        wq_moves = []
        wk_move = wv_move = None
        for i in range(3):
            j = len(ring_last)
            st = ring_reg(j)[:, 0:64]
            tr = nc.tensor.transpose(st, w_sb[:, i, :], id64)
            if i == 0:
                _after([tr], a0)
            wT = st.rearrange("p (h d) -> p h d", h=HPC)
            if i == 0:
                for h in range(HPC):
                    mv = nc.scalar.mul(
                        out=wq_pack4[:, h, h, 0:16], in_=wT[:, h, :], mul=SCALE_Q
                    )
                    if h == 0:
                        _dep(mv, tr)
                        _dep(mv, z0)
                    wq_moves.append(mv)
                ring_last.append(wq_moves[-1])
            elif i == 1:
                wk_move = nc.scalar.copy(out=wk_pack[:, :, 0:16], in_=wT)
                _dep(wk_move, tr)
                _dep(wk_move, z1)
                ring_last.append(wk_move)
            else:
                wv_move = nc.scalar.copy(out=wv_ext[:, :, 0:16], in_=wT)
                _dep(wv_move, tr)
                _dep(wv_move, z2)
                ring_last.append(wv_move)

        # jobs 3..34: x transposes (kv tiles first, then q tiles)
        xtr_copy = {}
        jobs = [(1, t) for t in range(NKT_FULL)] + [(0, t) for t in range(NKT_FULL)]
        for i, (g, t) in enumerate(jobs):
            j = len(ring_last)
            stage = ring_reg(j)[:, 0:128]
            tr = nc.tensor.transpose(stage, x_sb[:, g, t, :], identity)
            _dep(tr, (kv_dmas if g == 1 else q_dmas)[t // 4])
            if j >= 4:
                _dep(tr, ring_last[j - 4], reason="stage ring WAR")
            cp = nc.vector.tensor_copy(xT_sb[:, g, t, :], stage)
            _dep(cp, tr)
            xtr_copy[i] = cp
            ring_last.append(cp)

        xkv_flat = xT_sb[:, 1, :, :].rearrange("p t s -> p (t s)")
        xq_flat = xT_sb[:, 0, :, :].rearrange("p t s -> p (t s)")

        def proj_mm(lhsT, rhs, dst, dst_view=None, extra_deps=(), copy_eng="act"):
            j = len(ring_last)
            reg = ring_reg(j)
            mm = nc.tensor.matmul(
                reg[:, 0 : rhs.free_size()],
                lhsT=lhsT,
                rhs=rhs,
                start=True,
                stop=True,
            )
            if j >= 4:
                _dep(mm, ring_last[j - 4], reason="stage ring WAR")
            for d in extra_deps:
                _dep(mm, d)
            src = reg[:, 0 : rhs.free_size()]
            if dst_view is not None:
                src = dst_view(src)
            if copy_eng == "dve":
                cp = nc.vector.tensor_copy(dst, src)
            else:
                cp = nc.scalar.copy(out=dst, in_=src)
            _dep(cp, mm)
            ring_last.append(cp)
            return mm

        # jobs 35..38: kT projection (needs all kv copies: both engines)
        for ch in range(4):
            deps = (wk_move, xtr_copy[15]) if ch == 0 else ()
            proj_mm(
                wk_pack.rearrange("p h x -> p (h x)"),
                xkv_flat[:, bass.ts(ch, 512)],
                kT[:, bass.ts(ch, 512)],
                extra_deps=deps,
            )
        # jobs 39..54: v projection (lhsT = xT kv tile, self-load each)
        wv_flat = wv_ext.rearrange("p h x -> p (h x)")  # [P, 68]
        v_copies = []
        for kt in range(NKT_FULL):
            deps = (wv_move,) if kt == 0 else ()
            proj_mm(
                xT_sb[:, 1, kt, :],
                wv_flat,
                v_sb[:, kt, :, 0:17],
                dst_view=lambda src: src.rearrange("p (h x) -> p h x", h=HPC),
                extra_deps=deps,
                copy_eng="dve",
            )
            v_copies.append(ring_last[-1])
        ones = nc.vector.memset(v_sb[:, :, :, 16:17], 1.0)
        _dep(ones, v_copies[-1])
        # jobs 55..70: qTm projection (needs all q copies: both engines)
        for h in range(HPC):
            for ch in range(4):
                deps = [wq_moves[h]] if ch == 0 else []
                if h == 0 and ch == 0:
                    deps += [xtr_copy[31]]
                proj_mm(
                    wq_pack4[:, h, :, :].rearrange("p a b -> p (a b)"),
                    xq_flat[:, bass.ts(ch, 512)],
                    qTm[:, h, bass.ts(ch, 512)],
                    extra_deps=deps,
                )
        proj_copies = ring_last

        # pre-zero avbank: one full-bank matmul per bank writes 0 everywhere,
        # clears pending-zero state and sets every has_written bit, so ALL
        # main-loop AV matmuls are pure accumulates (start=False).
        for b in range(4):
            zmm = nc.tensor.matmul(
                avbank[:, 2 * b : 2 * b + 2, :].rearrange("p a b -> p (a b)"),
                lhsT=zmm_w,
                rhs=zmm_x,
                start=True,
                stop=True,
            )
            if b == 0:
                _dep(zmm, zw_ms)
                _dep(zmm, zx_ms)

        # ---------------- stage 1: main loop ----------------
        exp_done = {}
        last_av = {}
        first_S = None
        # anchor: last staging copies (qTm on ACT; v/xT on DVE)
        anchor = pe_abs(proj_copies[-1], v_copies[-1])

        pe_chain = [anchor]  # pin PE order to emission order (sync=False)

        def chain(mm):
            _after([mm], pe_chain[0])
            pe_chain[0] = mm

        for kt in range(NKT):
            eng = exp_assignment(kt)
            buf = kt % 2
            epos = {}
            na = nd = 0
            for c in range(NQB):
                if eng[c] == "act":
                    epos[c] = na
                    na += 1
                else:
                    epos[c] = nd
                    nd += 1
            s_mms = []
            for c in range(NQB):
                g = kt * NQB + c
                sl = g % 2
                for hb in range(2):
                    mm = nc.tensor.matmul(
                        sb[sl][:, 2 * hb : 2 * hb + 2, :].rearrange(
                            "p a b -> p (a b)"
                        ),
                        lhsT=kT[:, bass.ts(kt, P)],
                        rhs=qTm[:, 2 * hb : 2 * hb + 2, bass.ts(c, QB)],
                        start=True,
                        stop=True,
                    )
                    chain(mm)
                    if hb == 0:
                        if g >= 2:
                            _dep(mm, exp_done[g - 2], reason="ring WAR")
                        elif g == 0:
                            _dep(mm, anchor)
                            first_S = mm
                    s_mms.append(mm)
                # exp for this chunk
                src = sb[sl][:, :, :].rearrange("p a b -> p (a b)")
                etile = e_act[buf] if eng[c] == "act" else e_dve[buf]
                edst = etile[:, epos[c], :, :].rearrange("p a b -> p (a b)")
                if eng[c] == "act":
                    e = nc.scalar.activation(
                        edst,
                        src,
                        mybir.ActivationFunctionType.Exp,
                        bias=zbias,
                        scale=LN2,
                    )
                else:
                    e = nc.vector.tensor_scalar(
                        out=edst.bitcast(U16),
                        in0=src,
                        scalar1=1024.0,
                        scalar2=SCH_BIAS,
                        op0=mybir.AluOpType.mult,
                        op1=mybir.AluOpType.add,
                    )
                    if os.environ.get("V3_CLAMP") == "1":
                        # sim-only: HW's fp32->uint16 cast saturates negatives
                        # to 0; the interp wraps them. Clamp to match HW.
                        e2 = nc.vector.tensor_scalar_max(edst, edst, 0.0)
                        _dep(e2, e)
                        e = e2
                _dep(e, s_mms[-1], reason="scores ready")
                exp_done[g] = e
            # AV burst for this kt
            for c in range(NQB):
                g = kt * NQB + c
                etile = e_act[buf] if eng[c] == "act" else e_dve[buf]
                for h in range(HPC):
                    mm = nc.tensor.matmul(
                        avbank[32 * h : 32 * h + 32, c, :],
                        lhsT=v_sb[:, kt, h, :],
                        rhs=etile[:, epos[c], h, :],
                        start=False,
                        stop=(kt == NKT - 1),
                        skip_group_check=True,
                        tile_position=(0, 32 * h),
                    )
                    chain(mm)
                    if h == 0:
                        _dep(mm, exp_done[g], reason="e ready")
                        if kt == 0 and c == 0:
                            _dep(mm, ones, reason="v ready")
                    last_av[c] = mm

        # ---------------- stage 2: epilogue ----------------
        ocopies = {}
        for qb in range(NQB):
            if qb % 2 == 0:
                cp = nc.scalar.copy(
                    out=o_ev[:, qb // 2, :], in_=avbank[:, qb, :]
                )
            else:
                cp = nc.vector.tensor_copy(o_od[:, qb // 2, :], avbank[:, qb, :])
            # qb and qb|1 share a physical PSUM bank: wait for the bank's
            # last AV matmul (PE in-order, so it covers the pair)
            _dep(cp, last_av[qb | 1])
            ocopies[qb] = cp
        e1x = pe_abs(
            ocopies[0], exp_done[NCH - 1], exp_done[NCH - 2], after=pe_chain[0]
        )
        pe_chain[0] = e1x
        prev_mults = {}
        for qb in range(NQB):
            sl = qb % 2
            otile = o_ev if qb % 2 == 0 else o_od
            trs = []
            for sub in range(2):
                tr = nc.tensor.transpose(
                    sb[sl][:, sub, 0:128],
                    otile[:, qb // 2, bass.ts(sub, P)],
                    identity,
                )
                chain(tr)
                trs.append(tr)
            _after(trs, e1x)
            _dep(trs[0], ocopies[qb])
            if qb >= 2:
                _dep(trs[0], prev_mults[qb - 2], reason="slot WAR")
            rin = (
                sb[sl][:, 0:2, 0:128]
                .rearrange("p s (h x) -> p s h x", h=HPC)[:, :, :, 16]
            )
            rc = nc.vector.reciprocal(r_sb[:, qb, :, :], rin)
            _dep(rc, trs[-1])
            oin = (
                sb[sl][:, 0:2, 0:128]
                .rearrange("p s (h x) -> p s h x", h=HPC)[:, :, :, 0:16]
            )
            rb = r_sb[:, qb, :, :].to_broadcast((P, 2, HPC, 16))
            mult = nc.vector.tensor_tensor(
                out=ofin[:, qb, :, :].rearrange("p s (h d) -> p s h d", h=HPC),
                in0=oin,
                in1=rb,
                op=mybir.AluOpType.mult,
            )
            _dep(mult, rc)
            prev_mults[qb] = mult
            for sub in range(2):
                dma = nc.sync.dma_start(
                    out=out_d[qb * QB + sub * P : qb * QB + (sub + 1) * P, :],
                    in_=ofin[:, qb, sub, :],
                )
                _dep(dma, mult)
        if DUMP:
            dump_kT = sbig.tile([P, KS], F32)
            dump_qT = sbig.tile([P, HPC * Q], F32)
            dump_v = sbig.tile([P, NKT_FULL * HPC * 32], F32)
            dump_e = sbig.tile([P, 5 * HPC * QB], F32)
            dump_o = sbig.tile([P, 4 * QB], F32)
            cvs = [
                nc.vector.tensor_copy(dump_kT, kT),
                nc.vector.tensor_copy(dump_qT, qTm.rearrange("p a b -> p (a b)")),
                nc.vector.tensor_copy(dump_v, v_sb.rearrange("p a b c -> p (a b c)")),
                nc.vector.tensor_copy(dump_e, e_act[1].rearrange("p a b c -> p (a b c)")),
                nc.vector.tensor_copy(dump_o, o_ev.rearrange("p a b -> p (a b)")),
            ]
            for cv in cvs:
                _dep(cv, prev_mults[NQB - 1])
            nc.sync.dma_start(out=dkT[:, :], in_=dump_kT)
            nc.sync.dma_start(out=dqT[:, :], in_=dump_qT)
            nc.sync.dma_start(out=dv[:, :], in_=dump_v)
            nc.sync.dma_start(out=de[:, :], in_=dump_e)
            nc.sync.dma_start(out=do[:, :], in_=dump_o)
            nc.sync.dma_start(out=dr[:, :], in_=r_sb.rearrange("p a b c -> p (a b c)"))
    global _DBG
    _DBG = dict(kT=kT, qTm=qTm, v_sb=v_sb, r_sb=r_sb,
                ofin=ofin, xT_sb=xT_sb, wq_pack4=wq_pack4, wk_pack=wk_pack,
                wv_ext=wv_ext)
    _legalize_waits(nc)
    nc.compile()
    if os.environ.get("V3_DEDUP", "0") == "1":
        ndel = _dedup_ldweights(nc)
        if os.environ.get("V3_DEBUG"):
            print(f"v3: deduped {ndel} ldweights")
    return nc


_NC = None


def _get_nc():
    global _NC
    if _NC is None:
        _NC = build_attention_nc()
    return _NC


def make_in_maps(q_x, kv_x, w_q, w_k, w_v):
    q_x = np.asarray(q_x, dtype=np.float32)
    kv_x = np.asarray(kv_x, dtype=np.float32)
    w_q = np.asarray(w_q, dtype=np.float32)
    w_k = np.asarray(w_k, dtype=np.float32)
    w_v = np.asarray(w_v, dtype=np.float32)
    in_maps = []
    for core in range(N_CORES):
        b, hg = divmod(core, 2)
        rows = slice(hg * HPC * D, (hg + 1) * HPC * D)
        in_maps.append(
            {
                "qx": np.ascontiguousarray(q_x[b]),
                "kvx": np.ascontiguousarray(kv_x[b]),
                "wq": np.ascontiguousarray(w_q[rows]),
                "wk": np.ascontiguousarray(w_k[rows]),
                "wv": np.ascontiguousarray(w_v[rows]),
            }
        )
    return in_maps


def gather_out(results):
    out = np.empty((B, Q, H, D), dtype=np.float32)
    for core in range(N_CORES):
        b, hg = divmod(core, 2)
        out[b, :, hg * HPC : (hg + 1) * HPC, :] = results[core]["out"].reshape(
            Q, HPC, D
        )
    return out


def run(q_x, kv_x, w_q, w_k, w_v, **run_kwargs):
    nc = _get_nc()
    in_maps = make_in_maps(q_x, kv_x, w_q, w_k, w_v)
    res = run_bass_kernel_spmd(nc, in_maps, list(range(N_CORES)), **run_kwargs)
    return gather_out(res.results), res


def kernel(q_x, kv_x, w_q, w_k, w_v):
    out, _ = run(q_x, kv_x, w_q, w_k, w_v)
    return out


# revision 4
# speedup vs baseline: 1.0307x; 1.0307x over previous
"""Multi-head attention kernel for Trainium2 (Bass/Tile), 8-core SPMD — v3.

Problem: B=4, Q=K=2048, C=128, H=8, D=16 attention.
Sharding: core = (batch b, head-group hg): 4 batches x 2 groups of 4 heads.
Core output: out[b, :, 4*hg:4*hg+4, :] as [2048, 64].

v3 design (~205us vs v2 at ~374us):
  - Main loop batched per kt: 16 score MMs (2 per chunk: [2 heads, 256 q] =
    512 moving cols against the strip-packed kT tile; per-head separation
    via zero-strip masking), then 32 col-tiled AV MMs per kt.
  - Per-chunk exp with a 2-slot PSUM score ring: ACT (native exp) and DVE
    (Schraudolph: uint16(1024*x + bias) bitcast fp16; the saturating HW
    cast gives clean underflow->0) run concurrently on alternating chunks.
  - One PSUM tile PER ring slot and per (engine, kt-buffer) e tile: the
    tile framework tracks hazards at tile granularity, so tile-splitting
    is what lets S MMs / ACT exp / DVE exp actually pipeline.
  - AV PSUM protocol: pre-zero avbank with full-bank matmuls (start=True
    lazily zeroes the WHOLE 2KB bank), then all AV MMs accumulate with
    start=False. Epilogue drains wait the bank-pair's last AV (PSUM
    PE-write/engine-read same-bank collision is fatal).
  - Prologue: batched DMA (4 tiles per descriptor, sync+scalar queues),
    f32 PE transposes + single-engine fp16 copies through a bank-granular
    sbank staging ring whose final users fully rewrite each bank (no
    dangling PSUM pending-zero), projections after.
  - Epilogue: kt-outer tail, pipelined PE transposes + DVE recip/mult.

Known-bad on HW (do not re-enable blindly): post-compile LDWEIGHTS dedup
(V3_DEDUP=1) relies on PE weight residency that breaks under this
overlapped schedule despite passing CoreSim.
"""

import math
import os
import sys
from contextlib import ExitStack

import numpy as np

try:
    import concourse.bass as bass
except ImportError:  # container staging path
    sys.path.insert(0, "/opt/trn_rl_repo")
    import concourse.bass as bass

import concourse.bacc as bacc
import concourse.tile as tile
from concourse import mybir
from concourse.bass import _add_dep_helper
from concourse.bass_utils import run_bass_kernel_spmd

B, Q, KS, C, H, D = 4, 2048, 2048, 128, 8, 16
HPC = 4
N_CORES = 8
P = 128
QB = 256
NQB = Q // QB  # 8 chunks per kt
NKT_FULL = KS // P  # 16
NKT = int(os.environ.get("V3_NKT", NKT_FULL))  # truncated debug builds
NCH = NKT * NQB
F32 = mybir.dt.float32
F16 = mybir.dt.float16
U16 = mybir.dt.uint16
LN2 = math.log(2.0)
SCALE_Q = (1.0 / LN2) / math.sqrt(D)  # fold log2e into wq: scores in 2^x domain
ESHIFT = 2.0  # common 2^-ESHIFT factor: fp16/uint16 headroom (cancels in softmax)
SCH_BIAS = 15360.0 - 1024.0 * ESHIFT - 46.0  # fp16 exp bias + mid-rounding corr

# per-kt exp engine split: ACT ~997ns/chunk vs DVE ~1192ns/chunk
ACT_ONLY = os.environ.get("ACT_ONLY", "0") == "1"
_PROBE = []


def exp_assignment(kt):
    if ACT_ONLY:
        return ["act"] * NQB
    # 4.5 / 3.5 average split, alternating by kt parity
    if kt % 2 == 0:
        return ["act", "dve", "act", "dve", "act", "dve", "act", "act"]
    return ["act", "dve", "act", "dve", "act", "dve", "act", "dve"]


def _dep(inst, on, reason="dep"):
    _add_dep_helper(inst.ins, on.ins, sync=True, reason=reason)


def _after(insts, anchor, reason="order"):
    for i in insts:
        _add_dep_helper(i.ins, anchor.ins, sync=False, reason=reason)


def _legalize_waits(nc: bass.Bass) -> None:
    """Move excess embedded semaphore waits onto same-engine sequencer NOPs
    (TRN2 encodings carry at most one wait)."""
    nid = [0]
    for fn in nc.m.functions:
        for blk in fn.blocks:
            out = []
            changed = False
            for inst in blk.instructions:
                si = inst.sync_info
                if (
                    si is not None
                    and si.on_wait
                    and len(si.on_wait) > 1
                    and not (
                        inst.is_sequencer_only()
                        if callable(inst.is_sequencer_only)
                        else inst.is_sequencer_only
                    )
                ):
                    for w in si.on_wait:
                        nop = mybir.InstNoOp(name=f"W-{nid[0]}", ins=[], outs=[])
                        nid[0] += 1
                        nop.engine = inst.engine
                        nop.sync_info = mybir.SyncInfo(on_wait=[w], on_update=[])
                        nc.register_instruction(nop, overwrite=True)
                        out.append(nop)
                    inst.sync_info = mybir.SyncInfo(
                        on_wait=[], on_update=list(si.on_update)
                    )
                    changed = True
                out.append(inst)
            if changed:
                blk.instructions = out


def _dedup_ldweights(nc: bass.Bass) -> int:
    """Delete PE Ldweights that reload the exact weights already resident
    (same AP signature + tile position/size). HW keeps weights resident
    between matmuls; the paired Matmult instructions are non-self-loading
    after compile()'s split pass."""
    deleted = 0
    nid = [0]

    def colrange(pos, siz):
        if pos is None or siz is None:
            return (0, 128)
        return (pos[1], pos[1] + siz[1])

    for fn in nc.m.functions:
        for blk in fn.blocks:
            out = []
            resident = {}  # key: (pos, size) strings -> (ap sig, col range)
            for inst in blk.instructions:
                op = inst.concise_opcode()
                if op == "Ldweights":
                    pos = getattr(inst, "tile_position", None)
                    siz = getattr(inst, "tile_size", None)
                    key = (str(pos), str(siz))
                    cr = colrange(pos, siz)
                    sig = (
                        str(inst.ins[0]),
                        str(getattr(inst, "perf_mode", None)),
                        str(getattr(inst, "is_transpose", None)),
                    )
                    cur = resident.get(key)
                    if cur is not None and cur[0] == sig:
                        # redundant reload: drop, keep its sync on a NOP
                        si = inst.sync_info
                        if si is not None and (si.on_wait or si.on_update):
                            nop = mybir.InstNoOp(
                                name=f"LWD-{nid[0]}", ins=[], outs=[]
                            )
                            nid[0] += 1
                            nop.engine = inst.engine
                            nop.sync_info = si
                            nc.register_instruction(nop, overwrite=True)
                            out.append(nop)
                        deleted += 1
                        continue
                    # new load: invalidate entries whose col range overlaps
                    resident = {
                        k: v
                        for k, v in resident.items()
                        if v[1][1] <= cr[0] or v[1][0] >= cr[1]
                    }
                    resident[key] = (sig, cr)
                    out.append(inst)
                else:
                    out.append(inst)
            blk.instructions = out
    return deleted


def build_attention_nc() -> bass.Bass:
    nc = bacc.Bacc()
    qx_d = nc.dram_tensor("qx", [Q, C], F32, kind="ExternalInput")
    kvx_d = nc.dram_tensor("kvx", [KS, C], F32, kind="ExternalInput")
    wq_d = nc.dram_tensor("wq", [HPC * D, C], F32, kind="ExternalInput")
    wk_d = nc.dram_tensor("wk", [HPC * D, C], F32, kind="ExternalInput")
    wv_d = nc.dram_tensor("wv", [HPC * D, C], F32, kind="ExternalInput")
    out_d = nc.dram_tensor("out", [Q, HPC * D], F32, kind="ExternalOutput")
    DUMP = os.environ.get("V3_DUMP", "0") == "1"
    if DUMP:
        dkT = nc.dram_tensor("dkT", [P, KS], F32, kind="ExternalOutput")
        dqT = nc.dram_tensor("dqT", [P, HPC * Q], F32, kind="ExternalOutput")
        dv = nc.dram_tensor("dv", [P, NKT_FULL * HPC * 32], F32, kind="ExternalOutput")
        de = nc.dram_tensor("de", [P, 5 * HPC * QB], F32, kind="ExternalOutput")
        do = nc.dram_tensor("do", [P, 4 * QB], F32, kind="ExternalOutput")
        dr = nc.dram_tensor("dr", [P, NQB * 2 * HPC], F32, kind="ExternalOutput")

    with tile.TileContext(nc) as tc, ExitStack() as ctx:
        const = ctx.enter_context(tc.tile_pool(name="const", bufs=1))
        sbig = ctx.enter_context(tc.tile_pool(name="sbig", bufs=1))
        psum = ctx.enter_context(tc.tile_pool(name="psum", bufs=1, space="PSUM"))

        # ---- persistent PSUM: score ring (2 slots x 2 banks) + AV (4 banks) ----
        # One tile per ring slot: the tile framework tracks hazards at tile
        # granularity, so slot-separate tiles are what make the S/exp ring
        # actually pipeline (a single tile serializes S MMs vs both exps).
        sb4 = [psum.tile([P, 2, QB], F32, name=f"sb4_{i}") for i in range(4)]
        avbank = psum.tile([P, NQB, QB], F32)

        identity = const.tile([P, P], F32)
        id_ms = nc.gpsimd.memset(identity, 0.0)
        id_sel = nc.gpsimd.affine_select(
            out=identity,
            in_=identity,
            compare_op=mybir.AluOpType.not_equal,
            fill=1.0,
            base=0,
            pattern=[[-1, P]],
            channel_multiplier=1,
        )
        zbias = const.tile([P, 1], F32)
        nc.vector.memset(zbias, -ESHIFT * LN2)
        wabs = const.tile([1, 16], F16)  # PE LDW-absorber source
        nc.vector.memset(wabs, 0.0)
        zmm_w = const.tile([P, P], F16)  # zero stationary for avbank pre-zero
        zw_ms = nc.vector.memset(zmm_w, 0.0)
        zmm_x = const.tile([P, 512], F16)
        zx_ms = nc.vector.memset(zmm_x, 0.0)
        _abs_ctr = [0]

        def pe_abs(*waits, after=None):
            """Real-op absorber: standalone 1x1 fp16 LDWEIGHTS chain, one
            foreign tick per op (scheduler drops bare NOPs; identical APs get
            merged, so each absorber uses a unique source column). Safe only
            where the next matmul reloads its own weights; `after` pins the
            chain in PE program order so it cannot clobber resident weights."""
            ops = []
            for w in waits:
                i = _abs_ctr[0] % 16
                _abs_ctr[0] += 1
                ld = nc.tensor.ldweights(weights=wabs[0:1, i : i + 1])
                _dep(ld, w)
                if ops:
                    _after([ld], ops[-1])
                elif after is not None:
                    _after([ld], after)
                ops.append(ld)
            return ops[-1]

        # ---- SBUF ----
        x_sb = sbig.tile([P, 2, NKT_FULL, P], F32)  # [s, (q|kv), tile, c] staging
        xT_sb = sbig.tile([P, 2, NKT_FULL, P], F16)  # [c, (q|kv), tile, s]
        w_sb = sbig.tile([HPC * D, 3, C], F32)
        wq_pack4 = sbig.tile([P, HPC, HPC, 32], F16)  # [c, variant, strip, 32]
        wk_pack = sbig.tile([P, HPC, 32], F16)
        wv_ext = sbig.tile([P, HPC, 17], F16)
        qTm = sbig.tile([P, HPC, Q], F16)  # per-head masked, scaled log2e/sqrt(D)
        kT = sbig.tile([P, KS], F16)
        v_sb = sbig.tile([P, NKT_FULL, HPC, 32], F16)
        # e tiles: one per (engine, kt-buffer) so every tile has a single
        # writer engine (WAW stays on that engine's FIFO, ACT/DVE exps run
        # concurrently).
        e_act = [sbig.tile([P, 5, HPC, QB], F16, name=f"e_act{i}") for i in range(2)]
        e_dve = [sbig.tile([P, 4, HPC, QB], F16, name=f"e_dve{i}") for i in range(2)]
        o_ev = sbig.tile([P, 4, QB], F32)  # even qb drains (ACT)
        o_od = sbig.tile([P, 4, QB], F32)  # odd qb drains (DVE)
        r_sb = sbig.tile([P, NQB, 2, HPC], F32)
        ofin = sbig.tile([P, NQB, 2, HPC * D], F32)

        # ---------------- stage 0: DMA in (batched, 2 queues) ----------------
        wdmas = [
            nc.sync.dma_start(out=w_sb[:, 0, :], in_=wq_d[:, :]),
            nc.sync.dma_start(out=w_sb[:, 1, :], in_=wk_d[:, :]),
            nc.sync.dma_start(out=w_sb[:, 2, :], in_=wv_d[:, :]),
        ]
        # x DMAs: 4 tiles (512 rows) per descriptor; kv on sync, q on scalar
        kv_dmas = []
        q_dmas = []
        for i in range(4):
            src = kvx_d[512 * i : 512 * (i + 1), :].rearrange(
                "(t p) c -> p t c", p=P
            )
            kv_dmas.append(
                nc.sync.dma_start(out=x_sb[:, 1, 4 * i : 4 * i + 4, :], in_=src)
            )
        for i in range(4):
            src = qx_d[512 * i : 512 * (i + 1), :].rearrange("(t p) c -> p t c", p=P)
            q_dmas.append(
                nc.scalar.dma_start(out=x_sb[:, 0, 4 * i : 4 * i + 4, :], in_=src)
            )

        # zero-fill packed weight regions
        z0 = nc.vector.memset(wq_pack4, 0.0)
        z1 = nc.vector.memset(wk_pack, 0.0)
        z2 = nc.vector.memset(wv_ext, 0.0)
        z3 = nc.vector.memset(v_sb[:, :, :, 17:32], 0.0)

        # ---- weight transposes (avbank even-slot bank ring: banks 0,1,2) ----
        # ---- unified prologue PSUM staging ring over the 4 sbank banks ----
        # PSUM rules honored here:
        #  * PE-write + DVE/ACT-read of the SAME bank is fatal -> every ring
        #    region is one full bank, strictly sequenced producer (PE) ->
        #    consumer (copy) -> next producer (WAR dep 4 jobs back).
        #  * matmul start=True lazily zeroes its WHOLE bank; partial-bank
        #    staging leaves dangling pending-zero -> the ring's last users
        #    (kT/qTm projections) write full banks, and avbank is never
        #    staged on so the AV accumulation sees clean state.
        a0 = pe_abs(id_sel, wdmas[-1])
        id64 = identity[0 : HPC * D, 0 : HPC * D]
        ring_last = []  # per job: last consumer of that bank

        def ring_reg(j):
            return sb4[j % 4].rearrange("p a b -> p (a b)")

        # jobs 0..2: w transposes + packing moves (ACT)
        wq_moves = []
        wk_move = wv_move = None
        for i in range(3):
            j = len(ring_last)
            st = ring_reg(j)[:, 0:64]
            tr = nc.tensor.transpose(st, w_sb[:, i, :], id64)
            if i == 0:
                _after([tr], a0)
            wT = st.rearrange("p (h d) -> p h d", h=HPC)
            if i == 0:
                for h in range(HPC):
                    mv = nc.scalar.mul(
                        out=wq_pack4[:, h, h, 0:16], in_=wT[:, h, :], mul=SCALE_Q
                    )
                    if h == 0:
                        _dep(mv, tr)
                        _dep(mv, z0)
                    wq_moves.append(mv)
                ring_last.append(wq_moves[-1])
            elif i == 1:
                wk_move = nc.scalar.copy(out=wk_pack[:, :, 0:16], in_=wT)
                _dep(wk_move, tr)
                _dep(wk_move, z1)
                ring_last.append(wk_move)
            else:
                wv_move = nc.scalar.copy(out=wv_ext[:, :, 0:16], in_=wT)
                _dep(wv_move, tr)
                _dep(wv_move, z2)
                ring_last.append(wv_move)

        # jobs 3..34: x transposes (kv tiles first, then q tiles)
        xtr_copy = {}
        jobs = [(1, t) for t in range(NKT_FULL)] + [(0, t) for t in range(NKT_FULL)]
        for i, (g, t) in enumerate(jobs):
            j = len(ring_last)
            stage = ring_reg(j)[:, 0:128]
            tr = nc.tensor.transpose(stage, x_sb[:, g, t, :], identity)
            _dep(tr, (kv_dmas if g == 1 else q_dmas)[t // 4])
            if j >= 4:
                _dep(tr, ring_last[j - 4], reason="stage ring WAR")
            cp = nc.vector.tensor_copy(xT_sb[:, g, t, :], stage)
            _dep(cp, tr)
            xtr_copy[i] = cp
            ring_last.append(cp)

        xkv_flat = xT_sb[:, 1, :, :].rearrange("p t s -> p (t s)")
        xq_flat = xT_sb[:, 0, :, :].rearrange("p t s -> p (t s)")

        def proj_mm(lhsT, rhs, dst, dst_view=None, extra_deps=(), copy_eng="act"):
            j = len(ring_last)
            reg = ring_reg(j)
            mm = nc.tensor.matmul(
                reg[:, 0 : rhs.free_size()],
                lhsT=lhsT,
                rhs=rhs,
                start=True,
                stop=True,
            )
            if j >= 4:
                _dep(mm, ring_last[j - 4], reason="stage ring WAR")
            for d in extra_deps:
                _dep(mm, d)
            src = reg[:, 0 : rhs.free_size()]
            if dst_view is not None:
                src = dst_view(src)
            if copy_eng == "dve":
                cp = nc.vector.tensor_copy(dst, src)
            else:
                cp = nc.scalar.copy(out=dst, in_=src)
            _dep(cp, mm)
            ring_last.append(cp)
            return mm

        # jobs 35..38: kT projection (needs all kv copies: both engines)
        for ch in range(4):
            deps = (wk_move, xtr_copy[15]) if ch == 0 else ()
            proj_mm(
                wk_pack.rearrange("p h x -> p (h x)"),
                xkv_flat[:, bass.ts(ch, 512)],
                kT[:, bass.ts(ch, 512)],
                extra_deps=deps,
            )
        # jobs 39..54: v projection (lhsT = xT kv tile, self-load each)
        wv_flat = wv_ext.rearrange("p h x -> p (h x)")  # [P, 68]
        v_copies = []
        for kt in range(NKT_FULL):
            deps = (wv_move,) if kt == 0 else ()
            proj_mm(
                xT_sb[:, 1, kt, :],
                wv_flat,
                v_sb[:, kt, :, 0:17],
                dst_view=lambda src: src.rearrange("p (h x) -> p h x", h=HPC),
                extra_deps=deps,
                copy_eng="dve",
            )
            v_copies.append(ring_last[-1])
        ones = nc.vector.memset(v_sb[:, :, :, 16:17], 1.0)
        _dep(ones, v_copies[-1])
        # jobs 55..70: qTm projection (needs all q copies: both engines)
        for h in range(HPC):
            for ch in range(4):
                deps = [wq_moves[h]] if ch == 0 else []
                if h == 0 and ch == 0:
                    deps += [xtr_copy[31]]
                proj_mm(
                    wq_pack4[:, h, :, :].rearrange("p a b -> p (a b)"),
                    xq_flat[:, bass.ts(ch, 512)],
                    qTm[:, h, bass.ts(ch, 512)],
                    extra_deps=deps,
                )
        proj_copies = ring_last

        # pre-zero avbank: one full-bank matmul per bank writes 0 everywhere,
        # clears pending-zero state and sets every has_written bit, so ALL
        # main-loop AV matmuls are pure accumulates (start=False).
        for b in range(4):
            zmm = nc.tensor.matmul(
                avbank[:, 2 * b : 2 * b + 2, :].rearrange("p a b -> p (a b)"),
                lhsT=zmm_w,
                rhs=zmm_x,
                start=True,
                stop=True,
            )
            if b == 0:
                _dep(zmm, zw_ms)
                _dep(zmm, zx_ms)

        # ---------------- stage 1: main loop ----------------
        exp_done = {}
        exp_half = {}
        last_av = {}
        first_S = None
        # anchor: last staging copies (qTm on ACT; v/xT on DVE)
        anchor = pe_abs(proj_copies[-1], v_copies[-1])

        pe_chain = [anchor]  # pin PE order to emission order (sync=False)

        def chain(mm):
            _after([mm], pe_chain[0])
            pe_chain[0] = mm

        for kt in range(NKT):
            eng = exp_assignment(kt)
            buf = kt % 2
            epos = {}
            na = nd = 0
            for c in range(NQB):
                if eng[c] == "act":
                    epos[c] = na
                    na += 1
                else:
                    epos[c] = nd
                    nd += 1
            s_mms = []
            for c in range(NQB):
                g = kt * NQB + c
                for hb in range(2):
                    t4 = (2 * g + hb) % 4
                    mm = nc.tensor.matmul(
                        sb4[t4].rearrange("p a b -> p (a b)"),
                        lhsT=kT[:, bass.ts(kt, P)],
                        rhs=qTm[:, 2 * hb : 2 * hb + 2, bass.ts(c, QB)],
                        start=True,
                        stop=True,
                    )
                    chain(mm)
                    if g >= 2:
                        _dep(mm, exp_half[(g - 2, hb)], reason="ring WAR")
                    elif g == 0 and hb == 0:
                        _dep(mm, anchor)
                        first_S = mm
                    s_mms.append(mm)
                    # half-chunk exp: heads 2hb..2hb+1
                    src = sb4[t4].rearrange("p a b -> p (a b)")
                    etile = e_act[buf] if eng[c] == "act" else e_dve[buf]
                    edst = etile[:, epos[c], 2 * hb : 2 * hb + 2, :].rearrange(
                        "p a b -> p (a b)"
                    )
                    if eng[c] == "act":
                        e = nc.scalar.activation(
                            edst,
                            src,
                            mybir.ActivationFunctionType.Exp,
                            bias=zbias,
                            scale=LN2,
                        )
                    else:
                        e = nc.vector.tensor_scalar(
                            out=edst.bitcast(U16),
                            in0=src,
                            scalar1=1024.0,
                            scalar2=SCH_BIAS,
                            op0=mybir.AluOpType.mult,
                            op1=mybir.AluOpType.add,
                        )
                        if os.environ.get("V3_CLAMP") == "1":
                            e2 = nc.vector.tensor_scalar_max(edst, edst, 0.0)
                            _dep(e2, e)
                            e = e2
                    _dep(e, mm, reason="scores ready")
                    exp_half[(g, hb)] = e
                exp_done[g] = exp_half[(g, 1)]
            # AV burst for this kt
            for c in range(NQB):
                g = kt * NQB + c
                etile = e_act[buf] if eng[c] == "act" else e_dve[buf]
                for h in range(HPC):
                    mm = nc.tensor.matmul(
                        avbank[32 * h : 32 * h + 32, c, :],
                        lhsT=v_sb[:, kt, h, :],
                        rhs=etile[:, epos[c], h, :],
                        start=False,
                        stop=(kt == NKT - 1),
                        skip_group_check=True,
                        tile_position=(0, 32 * h),
                    )
                    chain(mm)
                    if h == 0:
                        _dep(mm, exp_done[g], reason="e ready")
                        if kt == 0 and c == 0:
                            _dep(mm, ones, reason="v ready")
                    last_av[c] = mm

        # ---------------- stage 2: epilogue ----------------
        ocopies = {}
        for qb in range(NQB):
            if qb % 2 == 0:
                cp = nc.scalar.copy(
                    out=o_ev[:, qb // 2, :], in_=avbank[:, qb, :]
                )
            else:
                cp = nc.vector.tensor_copy(o_od[:, qb // 2, :], avbank[:, qb, :])
            # qb and qb|1 share a physical PSUM bank: wait for the bank's
            # last AV matmul (PE in-order, so it covers the pair)
            _dep(cp, last_av[qb | 1])
            ocopies[qb] = cp
        e1x = pe_abs(
            ocopies[0], exp_done[NCH - 1], exp_done[NCH - 2], after=pe_chain[0]
        )
        pe_chain[0] = e1x
        prev_mults = {}
        for qb in range(NQB):
            t4 = qb % 4
            otile = o_ev if qb % 2 == 0 else o_od
            trs = []
            for sub in range(2):
                tr = nc.tensor.transpose(
                    sb4[t4][:, sub, 0:128],
                    otile[:, qb // 2, bass.ts(sub, P)],
                    identity,
                )
                chain(tr)
                trs.append(tr)
            _after(trs, e1x)
            _dep(trs[0], ocopies[qb])
            if qb >= 4:
                _dep(trs[0], prev_mults[qb - 4], reason="slot WAR")
            rin = (
                sb4[t4][:, 0:2, 0:128]
                .rearrange("p s (h x) -> p s h x", h=HPC)[:, :, :, 16]
            )
            rc = nc.vector.reciprocal(r_sb[:, qb, :, :], rin)
            _dep(rc, trs[-1])
            oin = (
                sb4[t4][:, 0:2, 0:128]
                .rearrange("p s (h x) -> p s h x", h=HPC)[:, :, :, 0:16]
            )
            rb = r_sb[:, qb, :, :].to_broadcast((P, 2, HPC, 16))
            mult = nc.vector.tensor_tensor(
                out=ofin[:, qb, :, :].rearrange("p s (h d) -> p s h d", h=HPC),
                in0=oin,
                in1=rb,
                op=mybir.AluOpType.mult,
            )
            _dep(mult, rc)
            prev_mults[qb] = mult
            for sub in range(2):
                dma = nc.sync.dma_start(
                    out=out_d[qb * QB + sub * P : qb * QB + (sub + 1) * P, :],
                    in_=ofin[:, qb, sub, :],
                )
                _dep(dma, mult)
        if DUMP:
            dump_kT = sbig.tile([P, KS], F32)
            dump_qT = sbig.tile([P, HPC * Q], F32)
            dump_v = sbig.tile([P, NKT_FULL * HPC * 32], F32)
            dump_e = sbig.tile([P, 5 * HPC * QB], F32)
            dump_o = sbig.tile([P, 4 * QB], F32)
            cvs = [
                nc.vector.tensor_copy(dump_kT, kT),
                nc.vector.tensor_copy(dump_qT, qTm.rearrange("p a b -> p (a b)")),
                nc.vector.tensor_copy(dump_v, v_sb.rearrange("p a b c -> p (a b c)")),
                nc.vector.tensor_copy(dump_e, e_act[1].rearrange("p a b c -> p (a b c)")),
                nc.vector.tensor_copy(dump_o, o_ev.rearrange("p a b -> p (a b)")),
            ]
            for cv in cvs:
                _dep(cv, prev_mults[NQB - 1])
            nc.sync.dma_start(out=dkT[:, :], in_=dump_kT)
            nc.sync.dma_start(out=dqT[:, :], in_=dump_qT)
            nc.sync.dma_start(out=dv[:, :], in_=dump_v)
            nc.sync.dma_start(out=de[:, :], in_=dump_e)
            nc.sync.dma_start(out=do[:, :], in_=dump_o)
            nc.sync.dma_start(out=dr[:, :], in_=r_sb.rearrange("p a b c -> p (a b c)"))
    global _DBG
    _DBG = dict(kT=kT, qTm=qTm, v_sb=v_sb, r_sb=r_sb,
                ofin=ofin, xT_sb=xT_sb, wq_pack4=wq_pack4, wk_pack=wk_pack,
                wv_ext=wv_ext)
    _legalize_waits(nc)
    nc.compile()
    if os.environ.get("V3_DEDUP", "0") == "1":
        ndel = _dedup_ldweights(nc)
        if os.environ.get("V3_DEBUG"):
            print(f"v3: deduped {ndel} ldweights")
    return nc


_NC = None


def _get_nc():
    global _NC
    if _NC is None:
        _NC = build_attention_nc()
    return _NC


def make_in_maps(q_x, kv_x, w_q, w_k, w_v):
    q_x = np.asarray(q_x, dtype=np.float32)
    kv_x = np.asarray(kv_x, dtype=np.float32)
    w_q = np.asarray(w_q, dtype=np.float32)
    w_k = np.asarray(w_k, dtype=np.float32)
    w_v = np.asarray(w_v, dtype=np.float32)
    in_maps = []
    for core in range(N_CORES):
        b, hg = divmod(core, 2)
        rows = slice(hg * HPC * D, (hg + 1) * HPC * D)
        in_maps.append(
            {
                "qx": np.ascontiguousarray(q_x[b]),
                "kvx": np.ascontiguousarray(kv_x[b]),
                "wq": np.ascontiguousarray(w_q[rows]),
                "wk": np.ascontiguousarray(w_k[rows]),
                "wv": np.ascontiguousarray(w_v[rows]),
            }
        )
    return in_maps


def gather_out(results):
    out = np.empty((B, Q, H, D), dtype=np.float32)
    for core in range(N_CORES):
        b, hg = divmod(core, 2)
        out[b, :, hg * HPC : (hg + 1) * HPC, :] = results[core]["out"].reshape(
            Q, HPC, D
        )
    return out


def run(q_x, kv_x, w_q, w_k, w_v, **run_kwargs):
    nc = _get_nc()
    in_maps = make_in_maps(q_x, kv_x, w_q, w_k, w_v)
    res = run_bass_kernel_spmd(nc, in_maps, list(range(N_CORES)), **run_kwargs)
    return gather_out(res.results), res


def kernel(q_x, kv_x, w_q, w_k, w_v):
    out, _ = run(q_x, kv_x, w_q, w_k, w_v)
    return out
